# revision 28
# baseline (speedup 1.0000x reference)
"""CLIP encoder layer (LN -> causal MHA -> residual -> LN -> quickGELU MLP -> residual)
on 8 Trainium2 NeuronCores, SPMD via bass/Tile.

Sharding: 8 shards = 4 batches x 2 parities. Core c handles batch c//2 with
parity s = c%2. Each core recomputes LN1 + K/V for the full sequence of its
batch (no collectives); the sequence stays in ORIGINAL token order for the
K/V path, so causal k-needs are prefixes of the k-tile order.

Own query rows are 4 chunks of 256 tokens, chosen so both parities need the
same total attention work under one shard-uniform program:
  s=0 chunks [7,4,3,0] need (16,10,8,2) k-tiles(128); s=1 [6,5,2,1] need
  (14,12,6,4). The program computes NG=(16,12,8,4) per group (position-wise
  max); surplus blocks are zeroed via mask DATA. Only the last 4 k-tiles of
  each group are mask-multiplied (they contain every partial/surplus block
  for both parities; all earlier k-tiles are fully below the diagonal).

Attention uses a transposed P.V: scores land [k, q] in PSUM, exp -> bf16 P,
then matmul(lhsT=P-slice, rhs=[V|1]) accumulates ctx token-major [q, hd+1];
column 64 is the softmax denominator per q-token (per PARTITION), so the
normalize is a cheap per-partition tensor_scalar instead of a broadcast
matmul. ctx is then PE-transposed to feature-major for the out-projection.

Everything flows bf16 (x, h1, K, Q, V, P, ctx, y, h2, weights); LN stats,
PSUM accum and the final output stay f32. All activations + K live in SBUF
(no DRAM spill). Weights are pre-permuted on the host into the exact SBUF
layout; LN affines, the 1/sqrt(hd) q-scale and quickGELU's 1.702 factor are
folded into weights/biases on the host.
"""

import sys

sys.path.insert(0, "/opt/trn_rl_repo")

import numpy as np
import ml_dtypes

import concourse.bass as bass
import concourse.mybir as mybir
import concourse.tile as tile
from concourse import bacc
from concourse.bass_utils import run_bass_kernel_spmd
from concourse.masks import make_identity

B, S, D = 4, 2048, 1024
H, HD = 16, 64
DFF = 4 * D
NCORES = 8
EPS = 1e-5
OWN = 1024
F32 = mybir.dt.float32
BF16 = mybir.dt.bfloat16
ALU = mybir.AluOpType
AF = mybir.ActivationFunctionType

CHUNKS = [[7, 4, 3, 0], [6, 5, 2, 1]]   # per-parity 256-token q-chunks
NG = [16, 12, 8, 4]                      # k-tiles(128) computed per q-group

_CACHE = {}
DEBUG_DUMPS = False   # when True, _build_program adds intermediate outputs


def _bcast(ap1d, p=128):
    """[N] DRAM vector viewed as [p, N] with partition-step 0 (broadcast load)."""
    return bass.AP(tensor=ap1d.tensor, offset=ap1d.offset, ap=[[0, p]] + list(ap1d.ap))


def _build_program():
    nc = bacc.Bacc("TRN2", target_bir_lowering=False, debug=False,
                   num_devices=NCORES)

    t = {}
    t["xb"] = nc.dram_tensor("xb", [S, D], BF16, kind="ExternalInput").ap()
    t["xq"] = nc.dram_tensor("xq", [OWN, D], BF16, kind="ExternalInput").ap()
    t["wk8"] = nc.dram_tensor("wk8", [8, 128, D], BF16, kind="ExternalInput").ap()
    t["wq8"] = nc.dram_tensor("wq8", [8, 128, D], BF16, kind="ExternalInput").ap()
    t["wvd"] = nc.dram_tensor("wvd", [128, 2, 8, 512], BF16, kind="ExternalInput").ap()
    t["wod"] = nc.dram_tensor("wod", [128, 8 * D], BF16, kind="ExternalInput").ap()
    t["w1t"] = nc.dram_tensor("w1t", [32, 128, D], BF16, kind="ExternalInput").ap()
    t["w2d"] = nc.dram_tensor("w2d", [128, 32 * D], BF16, kind="ExternalInput").ap()
    t["mskd"] = nc.dram_tensor("mskd", [128, 4, 4 * 256], BF16,
                               kind="ExternalInput").ap()
    t["bqd"] = nc.dram_tensor("bqd", [128, 8], F32, kind="ExternalInput").ap()
    t["bkd"] = nc.dram_tensor("bkd", [128, 8], F32, kind="ExternalInput").ap()
    t["b1d"] = nc.dram_tensor("b1d", [128, 32], F32, kind="ExternalInput").ap()
    for b_ in ("boh", "b2h"):
        t[b_] = nc.dram_tensor(b_, [D], BF16, kind="ExternalInput").ap()
    t["yo"] = nc.dram_tensor("yo", [OWN, D], F32, kind="ExternalOutput").ap()
    if DEBUG_DUMPS:
        for nm, shp in (("dbg_h1", [128, 8, S]), ("dbg_hq", [128, 8, OWN]),
                        ("dbg_vaug", [128, 16, H, 65]),
                        ("dbg_k", [128, 8, S]), ("dbg_q", [128, 8, OWN]),
                        ("dbg_ct0", [128, 2, H, 64]), ("dbg_ct1", [128, 2, H, 64]),
                        ("dbg_ct2", [128, 2, H, 64]), ("dbg_ct3", [128, 2, H, 64]),
                        ("dbg_y", [128, 8, OWN]), ("dbg_h2", [128, 8, OWN])):
            t[nm] = nc.dram_tensor(nm, shp, BF16, kind="ExternalOutput").ap()

    with tile.TileContext(nc) as tc:
        _body(nc, tc, t)
    nc.compile()
    return nc


def _ln_stats(nc, stat, eps_t, x_t, tag):
    """LayerNorm stats for one [128, D] token tile -> (nmu, rstd).
    Used for LN1 only (phase A), where the Sqrt activations cluster under a
    single sqrt-table load before the softmax Exp stream begins. LN2 defers
    its Sqrt to one batched op at MLP start (see p3_group / ln2_finish)."""
    st = stat.tile([128, 2, 6], F32, tag=f"{tag}st", name="st")
    for g in range(2):
        nc.vector.bn_stats(out=st[:, g, :], in_=x_t[:, g * 512:(g + 1) * 512])
    mv = stat.tile([128, 2], F32, tag=f"{tag}mv", name="mv")
    nc.vector.bn_aggr(out=mv, in_=st)
    rstd = stat.tile([128, 1], F32, tag=f"{tag}rs", name="rstd")
    nc.scalar.activation(out=rstd, in_=mv[:, 1:2], func=AF.Sqrt, bias=eps_t,
                         scale=1.0)
    nc.vector.reciprocal(out=rstd, in_=rstd)
    nmu = stat.tile([128, 1], F32, tag=f"{tag}nm", name="nmu")
    nc.vector.tensor_scalar(out=nmu, in0=mv[:, 0:1], scalar1=rstd, scalar2=-1.0,
                            op0=ALU.mult, op1=ALU.mult)
    return nmu, rstd


def _attn_group(nc, t, g, ast, m_hook=None, p3_hook=None):
    """Attention for q-group g, all 8 head-pairs.

    PSUM accumulation groups re-arm their whole 2KB bank on start (verified
    by HW probe), so concurrently-open groups must live in different banks
    and groups sharing a bank must be strictly sequential. Layout: pcx half
    per (m,hh) parity = one bank [128, 512]; within it qq=0 cols 0:65 and
    qq=1 cols 65:130 accumulate as two back-to-back groups. All P.V matmuls
    for one (m,hh) are emitted a full head behind its scores, so exp/mask
    latency is completely hidden."""
    k_sb, q_fm, vaug = ast["k_sb"], ast["q_fm"], ast["vaug"]
    mg = ast["mskp"].tile([128, 4, 256], BF16, tag="mg", name="mask_g")
    nc.gpsimd.dma_start(out=mg, in_=t["mskd"][:, g].rearrange(
        "p (k q) -> p k q", k=4))
    ctok = ast["ctxp"].tile([128, 2, H, 64], BF16, tag="ctok", name="ctok")
    ast["ctok"][g] = ctok
    ng = NG[g]
    nq = ng // 4
    order = [nq - 1] + list(range(nq - 1))  # masked quad first
    pend = []  # one entry per (m, hh): (pcx_half, pts, h)

    def flush(nkeep):
        while len(pend) > nkeep:
            pcxh, pts, h = pend.pop(0)
            for qq in range(2):
                reg = pcxh[:, qq * 65:(qq + 1) * 65]
                for ei, (kq, pt) in enumerate(pts):
                    for j in range(4):
                        nc.tensor.matmul(
                            reg, pt[:, j, qq * 128:(qq + 1) * 128],
                            vaug[:, 4 * kq + j, h, :],
                            start=(ei == 0 and j == 0),
                            stop=(ei == nq - 1 and j == 3),
                            skip_group_check=True)
                rec = ast["nrmp"].tile([128, 1], F32, tag="rec", name="rec")
                nc.vector.reciprocal(out=rec, in_=reg[:, 64:65])
                nc.vector.tensor_scalar(
                    out=ctok[:, qq, h, :], in0=reg[:, 0:64],
                    scalar1=rec, scalar2=None, op0=ALU.mult)

    for m in range(8):
        if m_hook is not None:
            m_hook(m)
        if p3_hook is not None and m == 3:
            p3_hook()
        for hh in range(2):
            h = 2 * m + hh
            hp = slice(hh * 64, (hh + 1) * 64)
            pcxh = ast["pcx_all"][:, (2 * m + hh) % 2]
            pts = []
            for kq in order:
                pst = ast["stps"].tile([128, 4, 256], F32, tag="st", name="pst")
                for j in range(4):
                    kt = 4 * kq + j
                    nc.tensor.matmul(
                        pst[:, j, :],
                        k_sb[hp, m, kt * 128:(kt + 1) * 128],
                        q_fm[hp, m, g * 256:(g + 1) * 256])
                pt = ast["ptp"].tile([128, 4, 256], BF16, tag="pt", name="pt")
                nc.scalar.activation(out=pt, in_=pst, func=AF.Exp)
                if kq == nq - 1:
                    nc.vector.tensor_mul(out=pt, in0=pt, in1=mg)
                pts.append((kq, pt))
            pend.append((pcxh, pts, h))
            flush(1)
    flush(0)


def _body(nc, tc, t):
    from contextlib import ExitStack
    with ExitStack() as _st:
        P = lambda *a, **k: _st.enter_context(tc.tile_pool(*a, **k))
        const = P(name="const", bufs=1)
        # PSUM budget is 8 banks. mmps (2) lives for the whole kernel; stps
        # (2x2) + cxps (2) are released after attention so the MLP can
        # double-buffer fc2 in their banks. PE transposes share stps's "st"
        # slots instead of a dedicated pool.
        mmps = P(name="mmps", bufs=2, space="PSUM")
        stps = tc.alloc_tile_pool(name="stps", bufs=2, space="PSUM")
        cxps = tc.alloc_tile_pool(name="cxps", bufs=1, space="PSUM")

        # attention-lifetime pools on the RIGHT side; released before the MLP
        ksbp = tc.alloc_tile_pool(name="ksbp", bufs=1, side="right")
        qfmp = tc.alloc_tile_pool(name="qfmp", bufs=1, side="right")
        vaugp = tc.alloc_tile_pool(name="vaugp", bufs=1, side="right")
        ctxp = tc.alloc_tile_pool(name="ctxp", bufs=2, side="right")
        mskp = tc.alloc_tile_pool(name="mskp", bufs=1, side="right")
        ptp = tc.alloc_tile_pool(name="ptp", bufs=6, side="right")
        nrmp = tc.alloc_tile_pool(name="nrmp", bufs=4, side="right")

        ident_b = const.tile([128, 128], BF16)
        make_identity(nc, ident_b)
        eps_t = const.tile([128, 1], F32)
        nc.vector.memset(eps_t, EPS)
        bq_t = const.tile([128, 8], F32)
        bk_t = const.tile([128, 8], F32)
        b1s_t = const.tile([128, 32], F32)
        bo_bc = const.tile([128, D], BF16)
        b2_bc = const.tile([128, D], BF16)

        k_sb = ksbp.tile([128, 8, S], BF16)
        q_fm = qfmp.tile([128, 8, OWN], BF16)
        vaug = vaugp.tile([128, 16, H, 65], BF16)
        nc.vector.memset(vaug[:, :, :, 64:65], 1.0)

        # one bank per rotation half (concurrent groups may not share a
        # bank on HW)
        pcx_all = cxps.tile([128, 2, 512], F32, name="pcx_all")
        ast = {"k_sb": k_sb, "q_fm": q_fm, "vaug": vaug, "mskp": mskp,
               "ctxp": ctxp, "ptp": ptp, "nrmp": nrmp, "stps": stps,
               "pcx_all": pcx_all, "ctok": {}}

        def transpose_to(src_t, dst_fm, tt, pool2=False):
            """[128, D] bf16 token-major tile -> 8 feature-major columns
            dst_fm[:, :, tt*128:(tt+1)*128] via one 8-transpose PSUM tile.
            Copies split Act + (Pool|DVE) to keep DVE free for LN work."""
            pt_ = stps.tile([128, 8, 128], BF16, tag="st", name="psT")
            for j in range(8):
                nc.tensor.transpose(
                    pt_[:, j, :], src_t[:, j * 128:(j + 1) * 128], ident_b)
            nc.scalar.copy(out=dst_fm[:, 0:4, tt * 128:(tt + 1) * 128],
                           in_=pt_[:, 0:4, :])
            nc.vector.tensor_copy(out=dst_fm[:, 4:8, tt * 128:(tt + 1) * 128],
                                  in_=pt_[:, 4:8, :])

        # ============ P0 + P1 scope (h1/hq/wv live here) ============
        with ExitStack() as _st2:
            P2 = lambda *a, **k: _st2.enter_context(tc.tile_pool(*a, **k))
            h1p = P2(name="h1p", bufs=1)
            hqp = P2(name="hqp", bufs=1)
            wvp = P2(name="wvp", bufs=1)
            xpool = P2(name="p0x", bufs=3)
            wrk0 = P2(name="p0w", bufs=2)
            stat0 = P2(name="p0stat", bufs=4)
            wstr = P2(name="wstr", bufs=2)

            h1_fm = h1p.tile([128, 8, S], BF16)
            hq_fm = hqp.tile([128, 8, OWN], BF16)
            wvh = wvp.tile([128, 2, 8, 512], BF16)

            def ln_tile(src_dram, row0, dst_fm, tt):
                x_t = xpool.tile([128, D], BF16, tag="x", name="x_t")
                nc.sync.dma_start(out=x_t, in_=src_dram[row0:row0 + 128, :])
                nmu, rstd = _ln_stats(nc, stat0, eps_t, x_t, "a")
                h1_t = wrk0.tile([128, D], BF16, tag="h1", name="h1_t")
                eng = nc.vector if tt % 2 == 0 else nc.gpsimd
                eng.tensor_scalar(out=h1_t, in0=x_t, scalar1=rstd,
                                  scalar2=nmu, op0=ALU.mult, op1=ALU.add)
                transpose_to(h1_t, dst_fm, tt, pool2=(tt % 2 == 1))

            # ---- P0: LN1 (xb then xq tiles) + V projection; K-projection
            # units for the lower half of the sequence are woven in as soon
            # as their h1 tiles exist, so PE never starves on the LN chain.
            def k_unit(m, qb, wkm):
                ps = mmps.tile([128, 512], F32, tag="mm", name="kps")
                for kt in range(8):
                    nc.tensor.matmul(
                        ps, wkm[:, kt, :],
                        h1_fm[:, kt, qb * 512:(qb + 1) * 512],
                        start=(kt == 0), stop=(kt == 7))
                if qb % 2 == 0:
                    nc.vector.tensor_scalar(
                        out=k_sb[:, m, qb * 512:(qb + 1) * 512], in0=ps,
                        scalar1=bk_t[:, m:m + 1], scalar2=None, op0=ALU.add)
                else:
                    nc.scalar.activation(
                        out=k_sb[:, m, qb * 512:(qb + 1) * 512], in_=ps,
                        func=AF.Identity, bias=bk_t[:, m:m + 1], scale=1.0)

            def q_unit(m, qb, wqm):
                ps = mmps.tile([128, 512], F32, tag="mm", name="qps")
                for kt in range(8):
                    nc.tensor.matmul(
                        ps, wqm[:, kt, :],
                        hq_fm[:, kt, qb * 512:(qb + 1) * 512],
                        start=(kt == 0), stop=(kt == 7))
                if qb % 2 == 0:
                    nc.vector.tensor_scalar(
                        out=q_fm[:, m, qb * 512:(qb + 1) * 512], in0=ps,
                        scalar1=bq_t[:, m:m + 1], scalar2=None, op0=ALU.add)
                else:
                    nc.scalar.activation(
                        out=q_fm[:, m, qb * 512:(qb + 1) * 512], in_=ps,
                        func=AF.Identity, bias=bq_t[:, m:m + 1], scale=1.0)

            nc.sync.dma_start(out=bq_t, in_=t["bqd"])
            nc.sync.dma_start(out=bk_t, in_=t["bkd"])
            wk_sb = {}
            woven = []  # (m, qb) K units in the tt loop: qb-major so units
            for qb in range(2):   # at tt only need h1 tiles 0..4(qb+1)-1
                for m in range(8):
                    woven.append((m, qb))
            for tt in range(S // 128):
                ln_tile(t["xb"], tt * 128, h1_fm, tt)
                if tt == 0:
                    # x(0) first in the queue, then the V-path constants
                    nc.sync.dma_start(out=wvh, in_=t["wvd"])
                if 2 <= tt < 10:
                    ln_tile(t["xq"], (tt - 2) * 128, hq_fm, tt - 2)
                for fb in range(2):
                    # V bias is folded into bo on the host (bv @ Wo), so the
                    # projection is a pure 8-matmul accumulation.
                    ps = mmps.tile([128, 512], F32, tag="mm", name="vps")
                    for kt in range(8):
                        nc.tensor.matmul(
                            ps, h1_fm[:, kt, tt * 128:(tt + 1) * 128],
                            wvh[:, fb, kt, :],
                            start=(kt == 0), stop=(kt == 7))
                    dst = vaug[:, tt, fb * 8:(fb + 1) * 8, 0:64]
                    srcv = ps.rearrange("p (h f) -> p h f", h=8)
                    if fb == 0:
                        nc.scalar.copy(out=dst, in_=srcv)
                    else:
                        nc.vector.tensor_copy(out=dst, in_=srcv)
                if tt == 3:
                    for m in range(8):
                        wkm = wstr.tile([128, 8, 128], BF16, tag="w",
                                        name="wkm", bufs=8)
                        nc.sync.dma_start(
                            out=wkm,
                            in_=t["wk8"][m].rearrange("p (t n) -> p t n", t=8))
                        wk_sb[m] = wkm
                if 4 <= tt < 12:
                    for m, qb in woven[(tt - 4) * 2:(tt - 4) * 2 + 2]:
                        k_unit(m, qb, wk_sb[m])

            # ---- rest of K + all of Q, attention group 0 interleaved ----
            def p1_m(m):
                k_unit(m, 2, wk_sb[m])
                k_unit(m, 3, wk_sb[m])
                wqm = wstr.tile([128, 8, 128], BF16, tag="w", name="wqm",
                                bufs=8)
                nc.sync.dma_start(
                    out=wqm, in_=t["wq8"][m].rearrange("p (t n) -> p t n", t=8))
                q_unit(m, 0, wqm)
                q_unit(m, 1, wqm)

            if DEBUG_DUMPS:
                nc.sync.dma_start(out=t["dbg_h1"], in_=h1_fm)
                nc.sync.dma_start(out=t["dbg_hq"], in_=hq_fm)
                nc.sync.dma_start(out=t["dbg_vaug"], in_=vaug)
            _attn_group(nc, t, 0, ast, m_hook=p1_m)
            if DEBUG_DUMPS:
                nc.sync.dma_start(out=t["dbg_k"], in_=k_sb)
                nc.sync.dma_start(out=t["dbg_q"], in_=q_fm)

        # ============ attention groups 1..3 + per-group epilogue ============
        with ExitStack() as _st3:
            P3 = lambda *a, **k: _st3.enter_context(tc.tile_pool(*a, **k))
            wop = P3(name="wop", bufs=1)
            ysbp = P3(name="ysbp", bufs=1)
            h2p = P3(name="h2p", bufs=1)
            w1str = P3(name="w1str", bufs=3)
            actp = P3(name="actp", bufs=2)   # all 4 a_gc cycle 2 slots
            mvp = P3(name="mvp", bufs=1)
            h2w = P3(name="h2w", bufs=1)
            lnf = P3(name="lnf", bufs=1)

            nc.gpsimd.dma_start(out=bo_bc, in_=_bcast(t["boh"]))
            wo_t = wop.tile([128, 8, D], BF16)
            nc.sync.dma_start(out=wo_t,
                              in_=t["wod"].rearrange("p (t n) -> p t n", t=8))
            y_sb = ysbp.tile([128, 8, OWN], BF16)
            h2sb = h2p.tile([128, 8, OWN], BF16)
            mv8 = mvp.tile([128, 8, 2], F32)

            with ExitStack() as _st3b:
                P3b = lambda *a, **k: _st3b.enter_context(tc.tile_pool(*a, **k))
                p3x = P3b(name="p3x", bufs=1)
                p3w = P3b(name="p3w", bufs=2)
                stat3 = P3b(name="p3stat", bufs=3)

                def p3_group(g):
                    """ctx transpose + out-proj + residual + LN2 for group g."""
                    ctok = ast["ctok"][g]
                    if DEBUG_DUMPS:
                        nc.sync.dma_start(out=t[f"dbg_ct{g}"], in_=ctok)
                    cfg = ctxp.tile([128, 8, 2, 128], BF16, tag="cfm", name="cfg", bufs=1)
                    for qq in range(2):
                        tqt = 2 * g + qq
                        ctv = ctok[:, qq].rearrange("p h f -> p (h f)")
                        pt_ = stps.tile([128, 8, 128], BF16, tag="st",
                                        name="psT3")
                        for j in range(8):
                            nc.tensor.transpose(
                                pt_[:, j, :], ctv[:, j * 128:(j + 1) * 128],
                                ident_b)
                        nc.vector.tensor_copy(out=cfg[:, 0:4, qq, :],
                                              in_=pt_[:, 0:4, :])
                        nc.scalar.copy(out=cfg[:, 4:8, qq, :],
                                       in_=pt_[:, 4:8, :])
                        x_t = p3x.tile([128, D], BF16, tag="x3", name="x3_t")
                        nc.gpsimd.dma_start(
                            out=x_t, in_=t["xq"][tqt * 128:(tqt + 1) * 128, :])
                        for n in range(2):
                            po = stps.tile([128, 512], F32, tag="st", name="po")
                            for kt in range(8):
                                nc.tensor.matmul(
                                    po, cfg[:, kt, qq, :],
                                    wo_t[:, kt, n * 512:(n + 1) * 512],
                                    start=(kt == 0), stop=(kt == 7))
                            yt = p3w.tile([128, 512], F32, tag="yt", name="yt")
                            nc.vector.tensor_add(
                                out=yt, in0=po,
                                in1=bo_bc[:, n * 512:(n + 1) * 512])
                            nc.gpsimd.tensor_add(
                                out=y_sb[:, tqt, n * 512:(n + 1) * 512],
                                in0=yt, in1=x_t[:, n * 512:(n + 1) * 512])
                        # LN2: only the DVE stats here; the Sqrt (a foreign
                        # act table) is batched at MLP start so the Exp
                        # stream never swaps tables.
                        st3 = stat3.tile([128, 2, 6], F32, tag="bst",
                                         name="st3")
                        for gg in range(2):
                            nc.vector.bn_stats(
                                out=st3[:, gg, :],
                                in_=y_sb[:, tqt, gg * 512:(gg + 1) * 512])
                        nc.vector.bn_aggr(out=mv8[:, tqt, :], in_=st3)

                # ---- LN2 finish + fc1 hoisting into the Act-bound
                # attention stream. LN2 rstds are produced in small batched
                # Sqrts (one table-swap pair each) as soon as their y tiles
                # exist; fc1 for token groups 0-1 then runs RAW (DVE evac,
                # no Silu table load) inside groups 1-3, filling PE bubbles
                # while Act grinds through the Exp stream.
                a_sl = {}
                lnp = {}

                def ln2_batch(lo, n):
                    rst = lnf.tile([128, n], F32, tag=f"rs{lo}", name="rst")
                    nc.scalar.activation(
                        out=rst,
                        in_=mv8[:, lo:lo + n, 1:2].rearrange("p t o -> p (t o)"),
                        func=AF.Sqrt, bias=eps_t, scale=1.0)
                    nc.vector.reciprocal(out=rst, in_=rst)
                    mrs = lnf.tile([128, n], F32, tag=f"ms{lo}", name="mrs")
                    nc.vector.tensor_mul(
                        out=mrs,
                        in0=mv8[:, lo:lo + n, 0:1].rearrange("p t o -> p (t o)"),
                        in1=rst)
                    lnp[lo] = (rst, mrs)

                def ln2_finish(tqt, lo):
                    rst, mrs = lnp[lo]
                    i = tqt - lo
                    h2_t = h2w.tile([128, D], BF16, tag="h2t", name="h2_t")
                    nc.vector.tensor_scalar(
                        out=h2_t, in0=y_sb[:, tqt, :],
                        scalar1=rst[:, i:i + 1], scalar2=mrs[:, i:i + 1],
                        op0=ALU.mult, op1=ALU.subtract)
                    pt_ = mmps.tile([128, 8, 128], BF16, tag="mm",
                                    name="psT4")
                    for j in range(8):
                        nc.tensor.transpose(
                            pt_[:, j, :], h2_t[:, j * 128:(j + 1) * 128],
                            ident_b)
                    nc.vector.tensor_copy(
                        out=h2sb[:, 0:4, tqt * 128:(tqt + 1) * 128],
                        in_=pt_[:, 0:4, :])
                    nc.scalar.copy(
                        out=h2sb[:, 4:8, tqt * 128:(tqt + 1) * 128],
                        in_=pt_[:, 4:8, :])

                def fc1_unit(gc, f, fused):
                    w1f = w1str.tile([128, 8, 128], BF16, tag="w1",
                                     name="w1f")
                    nc.sync.dma_start(
                        out=w1f,
                        in_=t["w1t"][f].rearrange("p (t n) -> p t n", t=8))
                    ps = mmps.tile([128, 256], F32, tag="mm", name="f1ps")
                    for kt in range(8):
                        nc.tensor.matmul(
                            ps, w1f[:, kt, :],
                            h2sb[:, kt, gc * 256:(gc + 1) * 256],
                            start=(kt == 0), stop=(kt == 7))
                    if fused:
                        nc.scalar.activation(out=a_sl[gc][:, f, :], in_=ps,
                                             func=AF.Silu, scale=1.702,
                                             bias=b1s_t[:, f:f + 1])
                    else:
                        nc.vector.tensor_copy(out=a_sl[gc][:, f, :], in_=ps)

                G2F = [0, 3, 6, 9, 12, 14, 16, 18, 20]  # g2's f-schedule

                def hoist_g1(m):
                    if m == 4:
                        ln2_batch(0, 2)
                        ln2_finish(0, 0)
                        ln2_finish(1, 0)
                        a_sl[0] = actp.tile([128, 32, 256], BF16, tag="a",
                                            name="a_g0")
                    elif m >= 5:
                        for f in range(4 * (m - 5), 4 * (m - 4)):
                            fc1_unit(0, f, False)

                def hoist_g2(m):
                    for f in range(12 + G2F[m], 12 + G2F[m + 1]):
                        fc1_unit(0, f, False)

                def hoist_g3(m):
                    if m == 0:
                        ln2_batch(2, 2)
                        ln2_finish(2, 2)
                        ln2_finish(3, 2)
                        a_sl[1] = actp.tile([128, 32, 256], BF16, tag="a",
                                            name="a_g1")
                    for f in range(4 * m, 4 * m + 4):
                        fc1_unit(1, f, False)

                hooks = {1: hoist_g1, 2: hoist_g2, 3: hoist_g3}
                for g in range(1, 4):
                    _attn_group(nc, t, g, ast, m_hook=hooks[g],
                                p3_hook=lambda g=g: p3_group(g - 1))
                p3_group(3)

            # attention done: free the right-side pools for the MLP, and the
            # attention PSUM pools so fc2 can multi-buffer in their banks
            for pool in (nrmp, ptp, mskp, ctxp, vaugp, qfmp, ksbp):
                pool.release()
            cxps.release()
            stps.release()

            # ============ MLP tail ============
            # fc1 for groups 0-1 already ran raw inside attention; here:
            # finish LN2 tiles 4-7, batch-Silu the raw activations (Silu
            # table loads once, after the Exp stream has ended -- gated by
            # b1s_t's write-after-read of y_sb tile 7), then fc2 +
            # fused-Silu fc1 for groups 2-3. fc2 multi-buffers in the banks
            # freed by the attention PSUM pools; w2 streams on the Pool DMA
            # queue so it doesn't contend with the w1 stream.
            with ExitStack() as _st4:
                P4 = lambda *a, **k: _st4.enter_context(tc.tile_pool(*a, **k))
                w2p = P4(name="w2p", bufs=1, side="right")
                f2ps = P4(name="f2ps", bufs=4, space="PSUM")
                outw = P4(name="outw", bufs=3)
                actp2 = P4(name="actp2", bufs=2)
                nc.vector.tensor_copy(out=b1s_t, in_=y_sb[:, 7, 0:32])
                nc.gpsimd.dma_start(out=b1s_t, in_=t["b1d"])
                nc.gpsimd.dma_start(out=b2_bc, in_=_bcast(t["b2h"]))
                # w2 rides the Act DMA queue: its two 12us transfers must not
                # sit in front of anything latency-critical on a shared
                # completion counter (on the Pool queue they stalled the
                # tile-7 residual adds and the whole LN2->fc1 chain ~20us).
                w2_t = w2p.tile([128, 32, D], BF16)
                w2v = t["w2d"].rearrange("p (t n) -> p t n", t=32)
                nc.scalar.dma_start(out=w2_t[:, :, 0:512], in_=w2v[:, :, 0:512])
                nc.scalar.dma_start(out=w2_t[:, :, 512:D], in_=w2v[:, :, 512:D])

                def silu_batch(gc):
                    a_gc = a_sl[gc]
                    for f in range(32):
                        nc.scalar.activation(out=a_gc[:, f, :],
                                             in_=a_gc[:, f, :], func=AF.Silu,
                                             scale=1.702,
                                             bias=b1s_t[:, f:f + 1])

                def fc1_c(gc, fused):
                    a_sl[gc] = actp2.tile([128, 32, 256], BF16, tag="a",
                                          name="a_gc")
                    for f in range(32):
                        fc1_unit(gc, f, fused)

                def fc2_gc(gc):
                    a_gc = a_sl[gc]
                    for t2 in range(2):
                        tqt = gc * 2 + t2
                        for n in range(2):
                            py = f2ps.tile([128, 512], F32, tag="f2",
                                           name="f2ps")
                            for kt in range(32):
                                nc.tensor.matmul(
                                    py, a_gc[:, kt, t2 * 128:(t2 + 1) * 128],
                                    w2_t[:, kt, n * 512:(n + 1) * 512],
                                    start=(kt == 0), stop=(kt == 31))
                            ot = outw.tile([128, 512], F32, tag="ot", name="ot")
                            nc.vector.tensor_add(
                                out=ot, in0=py,
                                in1=b2_bc[:, n * 512:(n + 1) * 512])
                            nc.gpsimd.tensor_add(
                                out=ot, in0=ot,
                                in1=y_sb[:, tqt, n * 512:(n + 1) * 512])
                            nc.sync.dma_start(
                                out=t["yo"][tqt * 128:(tqt + 1) * 128,
                                            n * 512:(n + 1) * 512],
                                in_=ot)

                if DEBUG_DUMPS:
                    nc.sync.dma_start(out=t["dbg_y"], in_=y_sb)
                    nc.sync.dma_start(out=t["dbg_h2"], in_=h2sb)
                # PE order: ln2 transposes -> fc1(2) -> fc2(0) -> fc2(1) ->
                # fc1(3) -> fc2(2) -> fc2(3); Act meanwhile: sqrt batch,
                # then silu batches 0-2 and fc1(3)'s fused evacs. fc1(2)
                # runs raw so its 27us of matmuls cover the silu batches
                # that gate fc2(0)/(1).
                ln2_batch(4, 4)
                for tqt in range(4, 8):
                    ln2_finish(tqt, 4)
                fc1_c(2, False)
                silu_batch(0)
                silu_batch(1)
                fc2_gc(0)
                silu_batch(2)
                fc2_gc(1)
                fc1_c(3, True)
                fc2_gc(2)
                fc2_gc(3)

def _perm_w_mtiles(W, mt):
    """[Din, Dout] -> [mt, 128, Din//128 * (Dout//mt)]."""
    din, dout = W.shape
    n_sz = dout // mt
    A = W.reshape(din // 128, 128, mt, n_sz)
    return np.ascontiguousarray(A.transpose(2, 1, 0, 3).reshape(mt, 128, -1))


def _perm_q(s):
    return np.concatenate([np.arange(256 * c, 256 * (c + 1)) for c in CHUNKS[s]])


def _masks(s):
    """[128, 4, 4*256] bf16: per q-group g, masks for its last 4 k-tiles."""
    m = np.zeros((4, 4, 128, 256), np.float32)
    for g in range(4):
        c = CHUNKS[s][g]
        q_orig = 256 * c + np.arange(256)
        for r in range(4):
            kt = NG[g] - 4 + r
            k_orig = kt * 128 + np.arange(128)
            m[g, r] = (k_orig[:, None] <= q_orig[None, :]).astype(np.float32)
    out = m.transpose(2, 0, 1, 3).reshape(128, 4, 4 * 256)
    return np.ascontiguousarray(out).astype(ml_dtypes.bfloat16)


def _prep_consts(inputs):
    f = {k: np.asarray(v, np.float64) for k, v in inputs.items()}
    g1, b1 = f["ln1_g"], f["ln1_b"]
    g2, b2 = f["ln2_g"], f["ln2_b"]
    qs = 1.0 / np.sqrt(HD)
    wq = ((g1[:, None] * f["Wq"]) * qs).astype(np.float32)
    wk = (g1[:, None] * f["Wk"]).astype(np.float32)
    wv = (g1[:, None] * f["Wv"]).astype(np.float32)
    w1 = (g2[:, None] * f["W1"]).astype(np.float32)
    bf = ml_dtypes.bfloat16
    c = {}
    c["wq8"] = _perm_w_mtiles(wq, 8).astype(bf)
    c["wk8"] = _perm_w_mtiles(wk, 8).astype(bf)
    c["wvd"] = np.ascontiguousarray(
        wv.reshape(8, 128, 2, 512).transpose(1, 2, 0, 3)).astype(bf)
    c["wod"] = np.ascontiguousarray(
        f["Wo"].astype(np.float32).reshape(8, 128, D).transpose(1, 0, 2)
        .reshape(128, 8 * D)).astype(bf)
    c["w1t"] = _perm_w_mtiles(w1, 32).astype(bf)
    c["w2d"] = np.ascontiguousarray(
        (f["W2"] / 1.702).astype(np.float32)
        .reshape(32, 128, D).transpose(1, 0, 2).reshape(128, 32 * D)).astype(bf)
    c["bqd"] = np.ascontiguousarray(
        ((b1 @ f["Wq"] + f["bq"]) * qs).astype(np.float32).reshape(8, 128).T)
    c["bkd"] = np.ascontiguousarray(
        (b1 @ f["Wk"] + f["bk"]).astype(np.float32).reshape(8, 128).T)
    c["b1d"] = np.ascontiguousarray(
        (1.702 * (b2 @ f["W1"] + f["b1"])).astype(np.float32).reshape(32, 128).T)
    # V bias (incl. LN1's beta pushed through Wv) folded through the out
    # projection: softmax rows sum to 1, so ctx = P.V + bv exactly once.
    bv_full = b1 @ f["Wv"] + f["bv"]
    c["boh"] = (f["bo"] + bv_full @ f["Wo"]).astype(bf)
    c["b2h"] = f["b2"].astype(bf)
    return c


def kernel(**inputs):
    if "nc" not in _CACHE:
        _CACHE["nc"] = _build_program()
        _CACHE["perms"] = [_perm_q(0), _perm_q(1)]
        _CACHE["masks"] = [_masks(0), _masks(1)]
    nc = _CACHE["nc"]
    perms, masks = _CACHE["perms"], _CACHE["masks"]

    x = np.asarray(inputs["x"], np.float32)
    c = _prep_consts(inputs)
    bf = ml_dtypes.bfloat16

    in_maps = []
    for core in range(NCORES):
        b, s = core // 2, core % 2
        m = dict(c)
        xb = x[b].astype(bf)
        m["xb"] = xb
        m["xq"] = np.ascontiguousarray(xb[perms[s]])
        m["mskd"] = masks[s]
        in_maps.append(m)

    res = run_bass_kernel_spmd(nc, in_maps, core_ids=list(range(NCORES)))

    out = np.empty((B, S, D), np.float32)
    for core in range(NCORES):
        b, s = core // 2, core % 2
        out[b][perms[s]] = res.results[core]["yo"]
    return out

